# revision 15
# baseline (speedup 1.0000x reference)
"""CTC loss (tf.keras ctc_batch_cost semantics) on 8 Trainium2 NeuronCores.

Strategy
--------
Data parallel: batch B=256 sharded 32 sequences per core; each core runs the
full CTC forward DP for its sequences, emits loss[32,1]; host concats.

Device algorithm
----------------
Probability-domain CTC forward recursion (NO log/exp in the hot loop):
    alpha[s,t] = q[s,t] * (alpha[s,t-1] + alpha[s-1,t-1] + r[s]*alpha[s-2,t-1])
with q[s,t] = y_pred[b,t,ext[s]] raw.  Range control is folded into the
cross-quadrant guard copies (every CTC path crosses each time boundary
exactly once, through a guard column): the paired Q0->Q1/Q2->Q3 copy scales
by 2^-RS_A, the Q1->Q2 copy by 2^-RS_B; the final op adds RS_TOT*ln2 back.
This keeps fp32 in range AND the final sum inside the ACT Ln domain (+-2^64)
with zero extra ops.

Per CTC row s the time recursion is an affine scan (d0 + state) * q = the
stock DVE `tensor_tensor_scan` (TTS).  Rows are swept s-major (wavefront).
T=512 is split into 4 time-quadrants of 128 columns on 4 partition blocks
(x32 seqs = 128 partitions).  Even rows are blanks sharing ONE [128,128]
blank tile as scan multiplier (halves DMA).  Odd rows build
d0 = alpha[s-1] + r*alpha[s-2] with one `scalar_tensor_tensor` (STT).

HW insight (measured): a chained TTS costs ~205ns + ~4.3ns/elem because the
scan drains serially and a dependent op must wait for it; ops from
INDEPENDENT chains interleaved on the engine hide that drain.  So each
quadrant's 128 columns are further split into NCHAIN staggered column-chains
(widths CW), each its own scan chain; consecutive DVE ops then belong to
different chains.  Chain j's scan-initial comes from chain j-1's last column
via a batched guard-column copy with DPS[j] rows of slack; chain 0's comes
from the previous quadrant (2 ACT ops per batch thanks to the [0,2,1,3]
quadrant->partition-block mapping; these carry the rescale).

Host side does only data movement/layout: label->column gather of y_pred and
packing the slot-skewed SBUF images.
"""

import numpy as np

# ---------------------------------------------------------------- constants
B, T, C = 256, 512, 512
L = 128
S = 2 * L + 1            # 257 extended CTC states
BLANK = C - 1
NCORE = 8
BG = B // NCORE          # 32 sequences per core
NCH = 4
CH = T // NCH            # 4 time quadrants of 128 columns each

# tunables (defaults; _build_nc args override) — HW-tuned 2026-08-08
CW = (64, 64)            # column-chain widths per quadrant (sum = CH)
DPS = (0, 4)             # stagger of chain j behind chain j-1 (even)
DQ = 4                   # cross-quadrant stagger (even, mult of GC)
GC = 2                   # cross-quadrant guard-copy batch
GH = 2                   # handoff-copy batch
RING = 32                # alpha ring slots (mult of GC and GH)
QSLAB = 8                # q DMA slab, in q slots
HOFF_ENG = "act"         # handoff copy engine: "act" | "dve"
RS_A = 26                # rescale 2^-RS_A at Q0->Q1 and Q2->Q3
RS_B = 25                # rescale 2^-RS_B at Q1->Q2
RS_TOT = 2 * RS_A + RS_B
LN2 = 0.6931471805599453

# quadrant k lives on partition block PB[k]*32 (guard copies: 2 ACT ops)
PB = [0, 2, 1, 3]

_CACHE = {}
TRACE = False
LAST_RES = None


def _geom(cw, dps, dq):
    """Chain offsets o[k][j], sizes."""
    nch = len(cw)
    assert sum(cw) == CH and len(dps) == nch and dps[0] == 0
    delta = sum(dps) + dq              # quadrant-to-quadrant skew
    o = [[k * delta + sum(dps[:j + 1]) for j in range(nch)] for k in range(NCH)]
    o_last = o[NCH - 1][nch - 1]
    nstep = S + o_last
    nq = (nstep - 1) // 2 + 1          # q/r slots (odd wavefront steps)
    wid = CH + nch                     # alpha row: guard+cols per chain
    gcol = [j + sum(cw[:j]) for j in range(nch)]     # guard col of chain j
    ccol = [g + 1 for g in gcol]                     # first data col
    qcol = [sum(cw[:j]) for j in range(nch)]         # q/blank col base
    return nch, o, o_last, nstep, nq, wid, gcol, ccol, qcol


def _build_nc(cw=None, dps=None, dq=None, gc=None, gh=None, ring=None,
              hoff=None, reps=1):
    cw = tuple(CW if cw is None else cw)
    dps = tuple(DPS if dps is None else dps)
    dq = DQ if dq is None else dq
    gc = GC if gc is None else gc
    gh = GH if gh is None else gh
    ring = RING if ring is None else ring
    hoff = HOFF_ENG if hoff is None else hoff

    import concourse.bacc as bacc
    import concourse.mybir as mybir
    import concourse.tile as tile

    f32 = mybir.dt.float32
    Alu = mybir.AluOpType
    Act = mybir.ActivationFunctionType

    nch, o, o_last, nstep, nq, wid, gcol, ccol, qcol = _geom(cw, dps, dq)
    assert dq % gc == 0 and ring % gc == 0 and dq % 2 == 0
    for d in dps[1:]:
        assert d % gh == 0 and d % 2 == 0 and d >= gh
    LC = wid - 1

    nc = bacc.Bacc("TRN2", target_bir_lowering=False, debug=False,
                   num_devices=NCORE)
    qin = nc.dram_tensor("qin", [128, nq * CH], f32, kind="ExternalInput")
    bln = nc.dram_tensor("bln", [128, CH], f32, kind="ExternalInput")
    rin = nc.dram_tensor("rin", [128, nch * nq], f32, kind="ExternalInput")
    loss_d = nc.dram_tensor("loss", [BG, 1], f32, kind="ExternalOutput")

    sl = lambda x: x % ring

    with tile.TileContext(nc) as tc:
        with tc.tile_pool(name="p", bufs=1) as pool:
            alpha = pool.tile([128, ring, wid], f32)
            qlab = pool.tile([128, nq, CH], f32)
            blank = pool.tile([128, CH], f32)
            rbuf = pool.tile([128, nch, nq], f32)
            ub = [pool.tile([128, cw[j]], f32, name=f"ub{j}")
                  for j in range(nch)]
            vbuf = pool.tile([BG, 1], f32)
            lnv = pool.tile([BG, 1], f32)
            lossb = pool.tile([BG, 1], f32)

            # --- init (re-emitted per rep; reps>1 is for benchmarking) ---
            for _rep in range(reps):
              half = ring // 2
              nc.vector.memset(alpha[:, 0:half, :], 0.0)
              nc.gpsimd.memset(alpha[:, half:ring, :], 0.0)
              nc.vector.memset(alpha[0:32, 0, 0:1], 1.0)   # alpha[0,-1] = 1
              if _rep == 0:
                nc.sync.dma_start(out=blank[:, :], in_=bln.ap()[:, :])
                nc.sync.dma_start(out=rbuf[:, :, :], in_=rin.ap()[:, :])
              nslab = (nq + QSLAB - 1) // QSLAB
              for i in range(nslab):
                s0 = i * QSLAB
                n = min(QSLAB, nq - s0)
                nc.sync.dma_start(
                    out=qlab[:, s0:s0 + n, :],
                    in_=qin.ap()[:, s0 * CH:(s0 + n) * CH],
                )

              # --- wavefront ---
              for w in range(nstep):
                psk = sl(w)
                if w % gc == 0:
                    sq = sl(w - dq)
                    # chain-0 guards from prev quadrant's last chain:
                    # Q0->Q1 + Q2->Q3 in one op (+64 partitions), * 2^-RS_A
                    nc.scalar.activation(
                        out=alpha[64:128, psk:psk + gc, 0],
                        in_=alpha[0:64, sq:sq + gc, LC],
                        func=Act.Copy, bias=0.0, scale=float(2.0 ** -RS_A),
                    )
                    # Q1->Q2 (the t=256 boundary), * 2^-RS_B
                    nc.scalar.activation(
                        out=alpha[32:64, psk:psk + gc, 0],
                        in_=alpha[64:96, sq:sq + gc, LC],
                        func=Act.Copy, bias=0.0, scale=float(2.0 ** -RS_B),
                    )
                if w % gh == 0:
                    for j in range(1, nch):
                        sq = sl(w - dps[j])
                        src = alpha[:, sq:sq + gh, gcol[j] - 1]   # prev chain end
                        dst = alpha[:, psk:psk + gh, gcol[j]]
                        if hoff == "act":
                            nc.scalar.copy(out=dst, in_=src)
                        else:
                            nc.vector.tensor_copy(out=dst, in_=src)
                odd = w % 2 == 1
                qs = w // 2
                if odd:
                    for j in range(nch):
                        nc.vector.scalar_tensor_tensor(
                            out=ub[j][:, :],
                            in0=alpha[:, sl(w - 2), gcol[j]:gcol[j] + cw[j]],
                            scalar=rbuf[:, j, qs:qs + 1],
                            in1=alpha[:, sl(w - 1), gcol[j]:gcol[j] + cw[j]],
                            op0=Alu.mult, op1=Alu.add,
                        )
                for j in range(nch):
                    if odd:
                        d0 = ub[j][:, :]
                        d1 = qlab[:, qs, qcol[j]:qcol[j] + cw[j]]
                    else:
                        d0 = alpha[:, sl(w - 1), gcol[j]:gcol[j] + cw[j]]
                        d1 = blank[:, qcol[j]:qcol[j] + cw[j]]
                    nc.vector.tensor_tensor_scan(
                        out=alpha[:, psk, ccol[j]:ccol[j] + cw[j]],
                        data0=d0, data1=d1,
                        initial=alpha[:, psk, gcol[j]:gcol[j] + 1],
                        op0=Alu.add, op1=Alu.mult,
                    )
                if w == 4:
                    # clear the alpha[0,-1]=1 seed before the ring wraps
                    nc.vector.memset(alpha[0:32, 0, 0:1], 0.0)

              # --- loss = -ln(a[S-1,T-1] + a[S-2,T-1]) - RS_TOT*ln2 ---
              w1 = S - 1 + o_last
              nc.vector.tensor_add(
                  out=vbuf[:, :],
                  in0=alpha[96:128, sl(w1), LC:LC + 1],
                  in1=alpha[96:128, sl(w1 - 1), LC:LC + 1],
              )
              nc.scalar.activation(out=lnv[:, :], in_=vbuf[:, :], func=Act.Ln)
              nc.vector.tensor_scalar(
                  out=lossb[:, :], in0=lnv[:, :],
                  scalar1=-1.0, scalar2=-float(RS_TOT) * LN2,
                  op0=Alu.mult, op1=Alu.add,
              )
              nc.sync.dma_start(out=loss_d.ap()[:, :], in_=lossb[:, :])

    nc.compile()
    return nc


def _host_prep(y_true, y_pred, cw=None, dps=None, dq=None):
    """Pure data movement: label->column gather + slot-skewed image packing."""
    cw = tuple(CW if cw is None else cw)
    dps = tuple(DPS if dps is None else dps)
    dq = DQ if dq is None else dq
    nch, o, o_last, nstep, nq, wid, gcol, ccol, qcol = _geom(cw, dps, dq)

    y_true = np.asarray(y_true).astype(np.int64)
    y_pred = np.ascontiguousarray(np.asarray(y_pred), dtype=np.float32)

    skipr = np.zeros((B, L), dtype=np.float32)
    skipr[:, 1:] = (y_true[:, 1:] != y_true[:, :-1]).astype(np.float32)

    # gather q for label rows: qg[b, r, t] = y_pred[b, t, y_true[b, r]]
    qg = np.empty((B, L, T), dtype=np.float32)
    for b in range(B):
        qg[b] = y_pred[b][:, y_true[b]].T
    qb_all = np.ascontiguousarray(y_pred[:, :, BLANK])   # [B, T]

    in_maps = []
    for core in range(NCORE):
        bs = slice(core * BG, (core + 1) * BG)
        qgc = qg[bs]          # [32, L, T]
        qbc = qb_all[bs]      # [32, T]
        skc = skipr[bs]       # [32, L]

        qimg = np.zeros((4, BG, nq, CH), dtype=np.float32)
        bimg = np.zeros((4, BG, CH), dtype=np.float32)
        rimg = np.zeros((4, BG, nch, nq), dtype=np.float32)
        for k in range(NCH):
            pb = PB[k]
            tsl = slice(k * CH, (k + 1) * CH)
            bimg[pb] = qbc[:, tsl]
            qk = qgc[:, :, tsl]               # [32, L, CH]
            for j in range(nch):
                oj2 = o[k][j] // 2
                cs = slice(qcol[j], qcol[j] + cw[j])
                qimg[pb, :, oj2:oj2 + L, cs] = qk[:, :, cs]
                rimg[pb, :, j, oj2:oj2 + L] = skc
        in_maps.append({
            "qin": qimg.reshape(128, nq * CH),
            "bln": bimg.reshape(128, CH),
            "rin": rimg.reshape(128, nch * nq),
        })
    return in_maps


def kernel(y_true, y_pred):
    from concourse import bass_utils

    if "nc" not in _CACHE:
        _CACHE["nc"] = _build_nc()
    nc = _CACHE["nc"]

    in_maps = _host_prep(y_true, y_pred)
    res = bass_utils.run_bass_kernel_spmd(nc, in_maps, core_ids=list(range(NCORE)),
                                          trace=TRACE)
    global LAST_RES
    LAST_RES = res
    out = np.concatenate([res.results[k]["loss"] for k in range(NCORE)], axis=0)
    return out.astype(np.float32)
